# revision 14
# baseline (speedup 1.0000x reference)
"""Trainium2 Bass kernel for nn_CELossWeighted_28698971472547.

Problem: weighted cross-entropy loss over x[16,32,256,256] logits with
target[16,256,256] class ids; per-pixel weight = 1/(global count of the
pixel's class); loss = sum(ce*w)/sum(w).

Data parallel over 8 NeuronCores (2 images per core). Per core:

  target broadcast (int8-packed, Pool engine):
    targets are packed on host to 1 byte/pixel: for each (tile, group)
    2048-px window, i16 element k = t[px k] | t[px 1024+k] << 8. Four
    tiles x 2048B are concatenated into one 8KB row per (block, group).
    GpSimd partition_broadcast (i32 view) replicates each row to the 32
    class partitions of its group - 16 calls total, cost ~free-size only.

  phase 1, channel-major tiles X [128=(4 groups x 32 ch), 2048 px]:
    ACT   E = exp(X) -> bf16
    DVE   u16[:, 0:1024]   = tb16 & 0xFF   (lo-byte pixels, 4x mode)
    DVE   u16[:, 1024:2048]= tb16 >> 8     (hi-byte pixels, 4x mode)
    DVE   oh = (u16 == iota_col) -> bf16, accum -> per-(g,c) counts (4x)
    DVE   ohe = oh * E (2x)
    PE    per 128-px chunk: lhsT = E-chunk / ohE-chunk [128,128],
          rhs = group-indicator [128,4]
          -> psum[pixel, (chunk, which, group)] = sumexp / exp(x_target)
  phase 2, pixel-major compact [128, 1024]:
    ACT   drain psum; logs = ln(sumexp); lesel = ln(exp(x_t)) ~= x_t
    DVE   v = 64*t + (logs - lesel) = 64*t + ce, stored fp16 (ce in
          [0,~14), v < 2048 so fp16 ulp <= 1; error averages out over
          ~4k px/class, well inside the 2e-2 gate)
    DVE   M_{c+1} = sum_p min(v, 64(c+1)) -- 32 clamp-accumulate passes
          at 4x fp16 rate
  host: fold per-core partials: per-class sums via the telescoping
        identity A_c = M_{c+1} - M_c - 64*N_{>c}; then
        loss = (sum_c A_c/count_c) / #classes-present.

Only lossless layout prep of the integer target happens on host (views,
byte packing of values 0..31) plus the final O(32) fold.
"""

import os
import sys

sys.path.insert(0, "/opt/trn_rl_repo")

ABLATE = int(os.environ.get("ABLATE", "0"))

from contextlib import ExitStack

import numpy as np
import ml_dtypes

import concourse.bass as bass  # noqa: F401
import concourse.tile as tile
from concourse import bacc, mybir
from concourse.bass_utils import run_bass_kernel_spmd

# Pin all activations (Exp/Ln/Copy) to the one table set that contains them
# all, so the table isn't re-loaded between interleaved Exp and Ln batches.
_orig_get_act_tables = bacc.get_activation_tables


def _pinned_act_tables(arch):
    tabs = dict(_orig_get_act_tables(arch))
    AFt = mybir.ActivationFunctionType
    pin = {AFt.Exp, AFt.Ln, AFt.Copy}
    out = {}
    for name, fs in tabs.items():
        if name == "natural_log_exp_and_others":
            out[name] = fs
        else:
            out[name] = fs - pin
    return out


bacc.get_activation_tables = _pinned_act_tables

BF16 = mybir.dt.bfloat16
F16 = mybir.dt.float16
F32 = mybir.dt.float32
I16 = mybir.dt.int16
I32 = mybir.dt.int32
AF = mybir.ActivationFunctionType
ALU = mybir.AluOpType

# ---- problem/shard geometry (hardcoded) ----
N_CORES = 8
C = 32
G = 4
CHUNK = 128
N_IMG = 2              # images per core
HWI = 256 * 256
T = 16                 # tiles per core
F = 2048               # pixels per (tile, group)
HF = F // 2            # pixels per byte lane
NCH = F // CHUNK       # 16 chunks per tile
PM_COLS = T * NCH * G  # 1024
TPB = 4                # tiles per broadcast block
NBLK = T // TPB        # 4 blocks
MSPLITS = 3
SPLITS = [(0, 8), (8, 13), (13, 16)]
MAXMIN_PER_TILE = 8
# h16 extraction engine per tile: True -> ACT (Copy scale=2^-8), False -> DVE
H16_ON_ACT = [t % 4 == 2 for t in range(T)]
# ohe hi-half product engine per tile: True -> Pool (gpsimd TT mult)
OHE_HI_ON_POOL = [True] * T
# ohe lo-half also on Pool for these tiles
OHE_LO_ON_POOL = [t % 3 == 1 for t in range(T)]
SHUF_I64 = False


def _build_nc():
    nc = bacc.Bacc("TRN2", target_bir_lowering=False, debug=False,
                   num_devices=N_CORES)
    x_d = nc.dram_tensor("x", [N_IMG, C, HWI], F32, kind="ExternalInput")
    # packed targets: one 8KB row per (group, block), as i32
    tpk_d = nc.dram_tensor("tpk", [G * NBLK, TPB * F // 4], I32,
                           kind="ExternalInput")
    tpm_d = nc.dram_tensor("tpm", [128, PM_COLS], BF16, kind="ExternalInput")
    blk4_d = nc.dram_tensor("blk4", [128, G], BF16, kind="ExternalInput")
    iota_d = nc.dram_tensor("iota", [128, 1], F32, kind="ExternalInput")
    out_d = nc.dram_tensor("out", [128, C + 1], F32, kind="ExternalOutput")

    with tile.TileContext(nc) as tc:
        with ExitStack() as ctx:
            _build_body(ctx, tc, x_d, tpk_d, tpm_d, blk4_d, iota_d, out_d)
    nc.compile()
    return nc


def _build_body(ctx, tc, x_d, tpk_d, tpm_d, blk4_d, iota_d, out_d):
    nc = tc.nc
    xap = x_d.ap()

    consts = ctx.enter_context(tc.tile_pool(name="consts", bufs=1))
    blk4 = consts.tile([128, G], BF16)
    nc.scalar.dma_start(blk4[:, :], blk4_d.ap())
    iota_col = consts.tile([128, 1], F32)
    nc.scalar.dma_start(iota_col[:, :], iota_d.ap())
    W4 = TPB * F // 4
    tsrc = consts.tile([128, W4], I32)
    for g in range(G):
        nc.sync.dma_start(tsrc[32 * g:32 * g + NBLK, :],
                          tpk_d.ap()[NBLK * g:NBLK * (g + 1), :])
    tpm = consts.tile([128, PM_COLS], BF16)
    nc.scalar.dma_start(tpm[:, :], tpm_d.ap())

    xpool = ctx.enter_context(tc.tile_pool(name="x", bufs=4))
    tbpool = ctx.enter_context(tc.tile_pool(name="tbp", bufs=2))
    epool = ctx.enter_context(tc.tile_pool(name="e", bufs=4))
    ppool = ctx.enter_context(tc.tile_pool(name="ps", bufs=1, space="PSUM"))

    se = consts.tile([128, 2 * PM_COLS], F32)
    cnt_cols = consts.tile([128, T], F32)
    out_sb = consts.tile([128, C + 1], F32)

    psum = ppool.tile([128, 4096], F32)  # bank t%8 = tile t

    ph2 = ctx.enter_context(tc.tile_pool(name="ph2", bufs=2))
    jpool = ctx.enter_context(tc.tile_pool(name="jp", bufs=2))
    mgr = consts.tile([128, C * MSPLITS], F32)

    prev_mm = None
    tb_blocks = [None] * NBLK
    pending = []
    vv_tiles = [None] * MSPLITS

    I64 = mybir.dt.int64

    def bcast_block(b):
        tb = tbpool.tile([128, W4], I32, tag="tb")
        if SHUF_I64:
            nc.vector.stream_shuffle(tb[:, :].bitcast(I64),
                                     tsrc[:, :].bitcast(I64), [b] * 32)
        else:
            nc.vector.stream_shuffle(tb[:, :], tsrc[:, :], [b] * 32)
        return tb

    tb_blocks[0] = bcast_block(0)

    for t in range(T):
        b, tau = divmod(t, TPB)
        if tau == 0 and b + 1 < NBLK:
            tb_blocks[b + 1] = bcast_block(b + 1)
        n = (G * t * F) // HWI
        off = (G * t * F) % HWI
        xt = xpool.tile([128, F], F32, tag="xt")
        nc.sync.dma_start(
            xt[:, :],
            xap[n][:, off:off + G * F].rearrange("c (g p) -> g c p", g=G))

        et = epool.tile([128, F], BF16, tag="et")
        nc.scalar.activation(et[:, :], xt[:, :], AF.Exp)

        if ABLATE >= 3:
            continue

        # byte extraction: tb16 element k of this tile = t[px k] | t[px 1024+k]<<8
        tbv = tb_blocks[b][:, tau * (F // 4):(tau + 1) * (F // 4)].bitcast(I16)
        u16 = epool.tile([128, F], I16, tag="u16")
        nc.vector.tensor_scalar(u16[:, 0:HF], tbv, 255, 0,
                                ALU.bitwise_and, ALU.bitwise_or)
        if H16_ON_ACT[t]:
            nc.scalar.activation(u16[:, HF:F], tbv, AF.Copy, scale=0.00390625)
        else:
            nc.vector.tensor_scalar(u16[:, HF:F], tbv, 8, 0,
                                    ALU.logical_shift_right, ALU.bitwise_or)
        oh = epool.tile([128, F], BF16, tag="oh")
        nc.vector.tensor_scalar(oh[:, :], u16[:, :], iota_col[:, 0:1], None,
                                ALU.is_equal, ALU.add,
                                accum_out=cnt_cols[:, t:t + 1])
        ohe = epool.tile([128, F], BF16, tag="ohe")
        lo_eng = nc.gpsimd if OHE_LO_ON_POOL[t] else nc.vector
        if OHE_HI_ON_POOL[t]:
            lo_eng.tensor_tensor(ohe[:, 0:HF], oh[:, 0:HF], et[:, 0:HF],
                                 ALU.mult)
            nc.gpsimd.tensor_tensor(ohe[:, HF:F], oh[:, HF:F], et[:, HF:F],
                                    ALU.mult)
        else:
            nc.vector.tensor_tensor(ohe[:, :], oh[:, :], et[:, :], ALU.mult)

        if ABLATE >= 2:
            continue
        for ch in range(NCH):
            base = (t % 8) * 512 + ch * 2 * G
            sl = slice(CHUNK * ch, CHUNK * (ch + 1))
            for (lo, src) in ((base, et), (base + G, ohe)):
                first = lo % 512 == 0
                last = lo % 512 == 504
                mm = nc.tensor.matmul(psum[:, lo:lo + G], src[:, sl],
                                      blk4[:, :], start=first, stop=last,
                                      skip_group_check=True)
                if prev_mm is not None:
                    tile.add_dep_helper(mm.ins, prev_mm.ins, sync=False,
                                        reason="psum bank program order")
                prev_mm = mm

        for dt in ([t - 1] if t < T - 1 else [t - 1, t]):
            if dt < 0:
                continue
            dsl = slice(dt * NCH * 2 * G, (dt + 1) * NCH * 2 * G)
            nc.scalar.activation(
                se[:, dsl],
                psum[:, (dt % 8) * 512:(dt % 8) * 512 + 128], AF.Copy)

        # incremental phase 2: prep each split when its drains are emitted,
        # then spread the 32 min-accum passes across later tile iterations
        done_ts = [t - 1] if t < T - 1 else [t - 1, t]
        ready = [i for i, (a, bb) in enumerate(SPLITS) if bb - 1 in done_ts]
        for sp in (ready if ABLATE < 1 else []):
            a, bb = SPLITS[sp]
            wt = PM_COLS // T
            w = (bb - a) * wt
            pm_sl = slice(a * wt, bb * wt)
            se_sl = se[:, 2 * a * wt: 2 * bb * wt]
            se_v = se_sl.rearrange("p (a w g) -> p a w g", w=2, g=G)
            logs = ph2.tile([128, w], F32, tag="logs")
            lesel = ph2.tile([128, w], F32, tag="lesel")
            nc.scalar.activation(logs[:, :].rearrange("p (a g) -> p a g", g=G),
                                 se_v[:, :, 0, :], AF.Ln)
            nc.scalar.activation(lesel[:, :].rearrange("p (a g) -> p a g", g=G),
                                 se_v[:, :, 1, :], AF.Ln)
            v1 = ph2.tile([128, w], F32, tag="v1")
            nc.vector.scalar_tensor_tensor(v1[:, :], tpm[:, pm_sl], 64.0,
                                           logs[:, :], ALU.mult, ALU.add)
            vv = ph2.tile([128, w], F16, tag=f"vv{sp}")
            nc.vector.tensor_tensor(vv[:, :], v1[:, :], lesel[:, :],
                                    ALU.subtract)
            vv_tiles[sp] = vv
            pending.extend((sp, c) for c in range(C))
        nmin = len(pending) if t == T - 1 else min(MAXMIN_PER_TILE,
                                                  len(pending))
        for _ in range(nmin):
            sp, c = pending.pop(0)
            a, bb = SPLITS[sp]
            w = (bb - a) * (PM_COLS // T)
            junk = jpool.tile([128, w], F16, tag="junk")
            nc.vector.tensor_scalar(
                junk[:, :], vv_tiles[sp][:, :], float(64 * (c + 1)), None,
                ALU.min, ALU.add,
                accum_out=mgr[:, c * MSPLITS + sp:c * MSPLITS + sp + 1])

    if ABLATE < 1:
        nc.vector.tensor_reduce(out_sb[:, 0:C],
                                mgr[:, :].rearrange("p (c s) -> p c s", s=MSPLITS),
                                mybir.AxisListType.X, ALU.add)
    else:
        nc.vector.memset(out_sb[:, 0:C], 0.0)
    nc.vector.tensor_reduce(out_sb[:, C:C + 1], cnt_cols[:, :],
                            mybir.AxisListType.X, ALU.add)
    nc.sync.dma_start(out_d.ap(), out_sb[:, :])


# ---- host side ----
def _pm_index():
    t_i, ch_i, g_i = np.meshgrid(np.arange(T), np.arange(NCH), np.arange(G),
                                 indexing="ij")
    col_pix = ((G * t_i + g_i) * F + CHUNK * ch_i).reshape(-1)
    return col_pix[None, :] + np.arange(CHUNK)[:, None]   # [128, PM_COLS]


_PM_IDX = _pm_index()
_BLK4 = np.zeros((128, G), dtype=ml_dtypes.bfloat16)
for _g in range(G):
    _BLK4[C * _g:C * (_g + 1), _g] = 1
_IOTA = np.tile(np.arange(C), G).reshape(128, 1).astype(np.float32)

_NC_CACHE = [None]


def _get_nc():
    if _NC_CACHE[0] is None:
        _NC_CACHE[0] = _build_nc()
    return _NC_CACHE[0]


def _pack_targets(tk):
    """tk: flat [2*HWI] int array for this core -> [G*NBLK, TPB*F//4] i32.

    Row (g, b) = concat over tau in [0,TPB) of the 2048-byte packed row
    for tile (TPB*b+tau), group g; within a tile-row, byte 2k holds
    t[px k] and byte 2k+1 holds t[px 1024+k]. Rows are ordered g-major
    so row (g, b) DMAs to SBUF partition 32*g+b for stream_shuffle."""
    tw = tk.reshape(T, G, F).astype(np.uint8)       # [t, g, px]
    packed = np.zeros((T, G, F), np.uint8)
    packed[:, :, 0::2] = tw[:, :, 0:HF]
    packed[:, :, 1::2] = tw[:, :, HF:F]
    # [t, g, F] -> [g, b, tau, F] -> rows (g*NBLK+b, tau*F)
    pb = packed.reshape(NBLK, TPB, G, F).transpose(2, 0, 1, 3)
    return np.ascontiguousarray(pb.reshape(G * NBLK, TPB * F)).view(np.int32)


def _make_in_maps(x, target):
    xs = np.asarray(x, dtype=np.float32).reshape(16, C, HWI)
    tf = np.asarray(target).reshape(16, HWI).astype(np.int32)
    in_maps = []
    for k in range(N_CORES):
        xk = np.ascontiguousarray(xs[2 * k:2 * k + 2])
        tk = np.ascontiguousarray(tf[2 * k:2 * k + 2]).reshape(-1)
        in_maps.append({
            "x": xk,
            "tpk": _pack_targets(tk),
            "tpm": tk[_PM_IDX].astype(ml_dtypes.bfloat16),
            "blk4": _BLK4,
            "iota": _IOTA,
        })
    return in_maps


def _fold(outs):
    M = np.zeros(C + 1, np.float64)   # M[0] = 0; M[j] = sum min(v, 64j)
    cnt = np.zeros(C, np.float64)
    for o in outs:
        o = np.asarray(o, dtype=np.float64)
        M[1:] += o[:, :C].sum(axis=0)
        cnt += o[:, C].reshape(G, C).sum(axis=0)
    n_gt = np.concatenate([np.cumsum(cnt[::-1])[::-1][1:], [0.0]])  # N_{>c}
    A = M[1:] - M[:-1] - 64.0 * n_gt
    present = cnt > 0
    num = (A[present] / cnt[present]).sum()
    den = float(present.sum())
    return np.float32(num / den)


def run_on_device(x, target, **run_kwargs):
    """Returns (loss, BassKernelResults)."""
    nc = _get_nc()
    in_maps = _make_in_maps(x, target)
    res = run_bass_kernel_spmd(nc, in_maps, core_ids=list(range(N_CORES)),
                               **run_kwargs)
    loss = _fold([res.results[k]["out"] for k in range(N_CORES)])
    return loss, res


def kernel(x, target):
    loss, _ = run_on_device(x, target)
    return loss


# revision 15
# speedup vs baseline: 1.0786x; 1.0786x over previous
"""Trainium2 Bass kernel for nn_CELossWeighted_28698971472547.

Problem: weighted cross-entropy loss over x[16,32,256,256] logits with
target[16,256,256] class ids; per-pixel weight = 1/(global count of the
pixel's class); loss = sum(ce*w)/sum(w).

Data parallel over 8 NeuronCores (2 images per core). Per core:

  target broadcast (int8-packed, Pool engine):
    targets are packed on host to 1 byte/pixel: for each (tile, group)
    2048-px window, i16 element k = t[px k] | t[px 1024+k] << 8. Four
    tiles x 2048B are concatenated into one 8KB row per (block, group).
    GpSimd partition_broadcast (i32 view) replicates each row to the 32
    class partitions of its group - 16 calls total, cost ~free-size only.

  phase 1, channel-major tiles X [128=(4 groups x 32 ch), 2048 px]:
    ACT   E = exp(X) -> bf16
    DVE   u16[:, 0:1024]   = tb16 & 0xFF   (lo-byte pixels, 4x mode)
    DVE   u16[:, 1024:2048]= tb16 >> 8     (hi-byte pixels, 4x mode)
    DVE   oh = (u16 == iota_col) -> bf16, accum -> per-(g,c) counts (4x)
    DVE   ohe = oh * E (2x)
    PE    per 128-px chunk: lhsT = E-chunk / ohE-chunk [128,128],
          rhs = group-indicator [128,4]
          -> psum[pixel, (chunk, which, group)] = sumexp / exp(x_target)
  phase 2, pixel-major compact [128, 1024]:
    ACT   drain psum; logs = ln(sumexp); lesel = ln(exp(x_t)) ~= x_t
    DVE   v = 64*t + (logs - lesel) = 64*t + ce, stored fp16 (ce in
          [0,~14), v < 2048 so fp16 ulp <= 1; error averages out over
          ~4k px/class, well inside the 2e-2 gate)
    DVE   M_{c+1} = sum_p min(v, 64(c+1)) -- 32 clamp-accumulate passes
          at 4x fp16 rate
  host: fold per-core partials: per-class sums via the telescoping
        identity A_c = M_{c+1} - M_c - 64*N_{>c}; then
        loss = (sum_c A_c/count_c) / #classes-present.

Only lossless layout prep of the integer target happens on host (views,
byte packing of values 0..31) plus the final O(32) fold.
"""

import os
import sys

sys.path.insert(0, "/opt/trn_rl_repo")

ABLATE = int(os.environ.get("ABLATE", "0"))

from contextlib import ExitStack

import numpy as np
import ml_dtypes

import concourse.bass as bass  # noqa: F401
import concourse.tile as tile
from concourse import bacc, mybir
from concourse.bass_utils import run_bass_kernel_spmd

# Pin all activations (Exp/Ln/Copy) to the one table set that contains them
# all, so the table isn't re-loaded between interleaved Exp and Ln batches.
_orig_get_act_tables = bacc.get_activation_tables


def _pinned_act_tables(arch):
    tabs = dict(_orig_get_act_tables(arch))
    AFt = mybir.ActivationFunctionType
    pin = {AFt.Exp, AFt.Ln, AFt.Copy}
    out = {}
    for name, fs in tabs.items():
        if name == "natural_log_exp_and_others":
            out[name] = fs
        else:
            out[name] = fs - pin
    return out


bacc.get_activation_tables = _pinned_act_tables

BF16 = mybir.dt.bfloat16
F16 = mybir.dt.float16
F32 = mybir.dt.float32
I16 = mybir.dt.int16
I32 = mybir.dt.int32
AF = mybir.ActivationFunctionType
ALU = mybir.AluOpType

# ---- problem/shard geometry (hardcoded) ----
N_CORES = 8
C = 32
G = 4
CHUNK = 128
N_IMG = 2              # images per core
HWI = 256 * 256
T = 16                 # tiles per core
F = 2048               # pixels per (tile, group)
HF = F // 2            # pixels per byte lane
NCH = F // CHUNK       # 16 chunks per tile
PM_COLS = T * NCH * G  # 1024
TPB = 4                # tiles per broadcast block
NBLK = T // TPB        # 4 blocks
MSPLITS = 3
SPLITS = [(0, 8), (8, 13), (13, 16)]
MAXMIN_PER_TILE = 8
# h16 extraction engine per tile: True -> ACT (Copy scale=2^-8), False -> DVE
H16_ON_ACT = [t % 4 == 2 for t in range(T)]
# ohe hi-half product engine per tile: True -> Pool (gpsimd TT mult)
OHE_HI_ON_POOL = [True] * T
# ohe lo-half also on Pool for these tiles
OHE_LO_ON_POOL = [False] * T
SHUF_I64 = False


def _build_nc():
    nc = bacc.Bacc("TRN2", target_bir_lowering=False, debug=False,
                   num_devices=N_CORES)
    x_d = nc.dram_tensor("x", [N_IMG, C, HWI], F32, kind="ExternalInput")
    # packed targets: one 8KB row per (group, block), as i32
    tpk_d = nc.dram_tensor("tpk", [G * NBLK, TPB * F // 4], I32,
                           kind="ExternalInput")
    tpm_d = nc.dram_tensor("tpm", [128, PM_COLS], BF16, kind="ExternalInput")
    blk4_d = nc.dram_tensor("blk4", [128, G], BF16, kind="ExternalInput")
    iota_d = nc.dram_tensor("iota", [128, 1], F32, kind="ExternalInput")
    out_d = nc.dram_tensor("out", [128, C + 1], F32, kind="ExternalOutput")

    with tile.TileContext(nc) as tc:
        with ExitStack() as ctx:
            _build_body(ctx, tc, x_d, tpk_d, tpm_d, blk4_d, iota_d, out_d)
    nc.compile()
    return nc


def _build_body(ctx, tc, x_d, tpk_d, tpm_d, blk4_d, iota_d, out_d):
    nc = tc.nc
    xap = x_d.ap()

    consts = ctx.enter_context(tc.tile_pool(name="consts", bufs=1))
    blk4 = consts.tile([128, G], BF16)
    nc.scalar.dma_start(blk4[:, :], blk4_d.ap())
    iota_col = consts.tile([128, 1], F32)
    nc.scalar.dma_start(iota_col[:, :], iota_d.ap())
    W4 = TPB * F // 4
    tsrc = consts.tile([128, W4], I32)
    for g in range(G):
        nc.sync.dma_start(tsrc[32 * g:32 * g + NBLK, :],
                          tpk_d.ap()[NBLK * g:NBLK * (g + 1), :])
    tpm = consts.tile([128, PM_COLS], BF16)
    nc.scalar.dma_start(tpm[:, :], tpm_d.ap())

    xpool = ctx.enter_context(tc.tile_pool(name="x", bufs=4))
    tbpool = ctx.enter_context(tc.tile_pool(name="tbp", bufs=2))
    epool = ctx.enter_context(tc.tile_pool(name="e", bufs=4))
    ppool = ctx.enter_context(tc.tile_pool(name="ps", bufs=1, space="PSUM"))

    se = consts.tile([128, 2 * PM_COLS], F32)
    cnt_cols = consts.tile([128, T], F32)
    out_sb = consts.tile([128, C + 1], F32)

    psum = ppool.tile([128, 4096], F32)  # bank t%8 = tile t

    ph2 = ctx.enter_context(tc.tile_pool(name="ph2", bufs=2))
    jpool = ctx.enter_context(tc.tile_pool(name="jp", bufs=2))
    mgr = consts.tile([128, C * MSPLITS], F32)

    prev_mm = None
    tb_blocks = [None] * NBLK
    pending = []
    vv_tiles = [None] * MSPLITS

    I64 = mybir.dt.int64

    def bcast_block(b):
        tb = tbpool.tile([128, W4], I32, tag="tb")
        if SHUF_I64:
            nc.vector.stream_shuffle(tb[:, :].bitcast(I64),
                                     tsrc[:, :].bitcast(I64), [b] * 32)
        else:
            nc.vector.stream_shuffle(tb[:, :], tsrc[:, :], [b] * 32)
        return tb

    tb_blocks[0] = bcast_block(0)

    for t in range(T):
        b, tau = divmod(t, TPB)
        if tau == 0 and b + 1 < NBLK:
            tb_blocks[b + 1] = bcast_block(b + 1)
        n = (G * t * F) // HWI
        off = (G * t * F) % HWI
        xt = xpool.tile([128, F], F32, tag="xt")
        nc.sync.dma_start(
            xt[:, :],
            xap[n][:, off:off + G * F].rearrange("c (g p) -> g c p", g=G))

        et = epool.tile([128, F], BF16, tag="et")
        nc.scalar.activation(et[:, :], xt[:, :], AF.Exp)

        if ABLATE >= 3:
            continue

        # byte extraction: tb16 element k of this tile = t[px k] | t[px 1024+k]<<8
        tbv = tb_blocks[b][:, tau * (F // 4):(tau + 1) * (F // 4)].bitcast(I16)
        u16 = epool.tile([128, F], I16, tag="u16")
        nc.vector.tensor_scalar(u16[:, 0:HF], tbv, 255, 0,
                                ALU.bitwise_and, ALU.bitwise_or)
        if H16_ON_ACT[t]:
            nc.scalar.activation(u16[:, HF:F], tbv, AF.Copy, scale=0.00390625)
        else:
            nc.vector.tensor_scalar(u16[:, HF:F], tbv, 8, 0,
                                    ALU.logical_shift_right, ALU.bitwise_or)
        oh = epool.tile([128, F], BF16, tag="oh")
        nc.vector.tensor_scalar(oh[:, :], u16[:, :], iota_col[:, 0:1], None,
                                ALU.is_equal, ALU.add,
                                accum_out=cnt_cols[:, t:t + 1])
        ohe = epool.tile([128, F], BF16, tag="ohe")
        lo_eng = nc.gpsimd if OHE_LO_ON_POOL[t] else nc.vector
        if OHE_HI_ON_POOL[t]:
            lo_eng.tensor_tensor(ohe[:, 0:HF], oh[:, 0:HF], et[:, 0:HF],
                                 ALU.mult)
            nc.gpsimd.tensor_tensor(ohe[:, HF:F], oh[:, HF:F], et[:, HF:F],
                                    ALU.mult)
        else:
            nc.vector.tensor_tensor(ohe[:, :], oh[:, :], et[:, :], ALU.mult)

        if ABLATE >= 2:
            continue
        for ch in range(NCH):
            base = (t % 8) * 512 + ch * 2 * G
            sl = slice(CHUNK * ch, CHUNK * (ch + 1))
            for (lo, src) in ((base, et), (base + G, ohe)):
                first = lo % 512 == 0
                last = lo % 512 == 504
                mm = nc.tensor.matmul(psum[:, lo:lo + G], src[:, sl],
                                      blk4[:, :], start=first, stop=last,
                                      skip_group_check=True)
                if prev_mm is not None:
                    tile.add_dep_helper(mm.ins, prev_mm.ins, sync=False,
                                        reason="psum bank program order")
                prev_mm = mm

        for dt in ([t - 1] if t < T - 1 else [t - 1, t]):
            if dt < 0:
                continue
            dsl = slice(dt * NCH * 2 * G, (dt + 1) * NCH * 2 * G)
            nc.scalar.activation(
                se[:, dsl],
                psum[:, (dt % 8) * 512:(dt % 8) * 512 + 128], AF.Copy)

        # incremental phase 2: prep each split when its drains are emitted,
        # then spread the 32 min-accum passes across later tile iterations
        done_ts = [t - 1] if t < T - 1 else [t - 1, t]
        ready = [i for i, (a, bb) in enumerate(SPLITS) if bb - 1 in done_ts]
        for sp in (ready if ABLATE < 1 else []):
            a, bb = SPLITS[sp]
            wt = PM_COLS // T
            w = (bb - a) * wt
            pm_sl = slice(a * wt, bb * wt)
            se_sl = se[:, 2 * a * wt: 2 * bb * wt]
            se_v = se_sl.rearrange("p (a w g) -> p a w g", w=2, g=G)
            logs = ph2.tile([128, w], F32, tag="logs")
            lesel = ph2.tile([128, w], F32, tag="lesel")
            nc.scalar.activation(logs[:, :].rearrange("p (a g) -> p a g", g=G),
                                 se_v[:, :, 0, :], AF.Ln)
            nc.scalar.activation(lesel[:, :].rearrange("p (a g) -> p a g", g=G),
                                 se_v[:, :, 1, :], AF.Ln)
            v1 = ph2.tile([128, w], F32, tag="v1")
            nc.vector.scalar_tensor_tensor(v1[:, :], tpm[:, pm_sl], 64.0,
                                           logs[:, :], ALU.mult, ALU.add)
            vv = ph2.tile([128, w], F16, tag=f"vv{sp}")
            nc.vector.tensor_tensor(vv[:, :], v1[:, :], lesel[:, :],
                                    ALU.subtract)
            vv_tiles[sp] = vv
            pending.extend((sp, c) for c in range(C))
        nmin = len(pending) if t == T - 1 else min(MAXMIN_PER_TILE,
                                                  len(pending))
        for _ in range(nmin):
            sp, c = pending.pop(0)
            a, bb = SPLITS[sp]
            w = (bb - a) * (PM_COLS // T)
            junk = jpool.tile([128, w], F16, tag="junk")
            nc.vector.tensor_scalar(
                junk[:, :], vv_tiles[sp][:, :], float(64 * (c + 1)), None,
                ALU.min, ALU.add,
                accum_out=mgr[:, c * MSPLITS + sp:c * MSPLITS + sp + 1])

    if ABLATE < 1:
        nc.vector.tensor_reduce(out_sb[:, 0:C],
                                mgr[:, :].rearrange("p (c s) -> p c s", s=MSPLITS),
                                mybir.AxisListType.X, ALU.add)
    else:
        nc.vector.memset(out_sb[:, 0:C], 0.0)
    nc.vector.tensor_reduce(out_sb[:, C:C + 1], cnt_cols[:, :],
                            mybir.AxisListType.X, ALU.add)
    nc.sync.dma_start(out_d.ap(), out_sb[:, :])


# ---- host side ----
def _pm_index():
    t_i, ch_i, g_i = np.meshgrid(np.arange(T), np.arange(NCH), np.arange(G),
                                 indexing="ij")
    col_pix = ((G * t_i + g_i) * F + CHUNK * ch_i).reshape(-1)
    return col_pix[None, :] + np.arange(CHUNK)[:, None]   # [128, PM_COLS]


_PM_IDX = _pm_index()
_BLK4 = np.zeros((128, G), dtype=ml_dtypes.bfloat16)
for _g in range(G):
    _BLK4[C * _g:C * (_g + 1), _g] = 1
_IOTA = np.tile(np.arange(C), G).reshape(128, 1).astype(np.float32)

_NC_CACHE = [None]


def _get_nc():
    if _NC_CACHE[0] is None:
        _NC_CACHE[0] = _build_nc()
    return _NC_CACHE[0]


def _pack_targets(tk):
    """tk: flat [2*HWI] int array for this core -> [G*NBLK, TPB*F//4] i32.

    Row (g, b) = concat over tau in [0,TPB) of the 2048-byte packed row
    for tile (TPB*b+tau), group g; within a tile-row, byte 2k holds
    t[px k] and byte 2k+1 holds t[px 1024+k]. Rows are ordered g-major
    so row (g, b) DMAs to SBUF partition 32*g+b for stream_shuffle."""
    tw = tk.reshape(T, G, F).astype(np.uint8)       # [t, g, px]
    packed = np.zeros((T, G, F), np.uint8)
    packed[:, :, 0::2] = tw[:, :, 0:HF]
    packed[:, :, 1::2] = tw[:, :, HF:F]
    # [t, g, F] -> [g, b, tau, F] -> rows (g*NBLK+b, tau*F)
    pb = packed.reshape(NBLK, TPB, G, F).transpose(2, 0, 1, 3)
    return np.ascontiguousarray(pb.reshape(G * NBLK, TPB * F)).view(np.int32)


def _make_in_maps(x, target):
    xs = np.asarray(x, dtype=np.float32).reshape(16, C, HWI)
    tf = np.asarray(target).reshape(16, HWI).astype(np.int32)
    in_maps = []
    for k in range(N_CORES):
        xk = np.ascontiguousarray(xs[2 * k:2 * k + 2])
        tk = np.ascontiguousarray(tf[2 * k:2 * k + 2]).reshape(-1)
        in_maps.append({
            "x": xk,
            "tpk": _pack_targets(tk),
            "tpm": tk[_PM_IDX].astype(ml_dtypes.bfloat16),
            "blk4": _BLK4,
            "iota": _IOTA,
        })
    return in_maps


def _fold(outs):
    M = np.zeros(C + 1, np.float64)   # M[0] = 0; M[j] = sum min(v, 64j)
    cnt = np.zeros(C, np.float64)
    for o in outs:
        o = np.asarray(o, dtype=np.float64)
        M[1:] += o[:, :C].sum(axis=0)
        cnt += o[:, C].reshape(G, C).sum(axis=0)
    n_gt = np.concatenate([np.cumsum(cnt[::-1])[::-1][1:], [0.0]])  # N_{>c}
    A = M[1:] - M[:-1] - 64.0 * n_gt
    present = cnt > 0
    num = (A[present] / cnt[present]).sum()
    den = float(present.sum())
    return np.float32(num / den)


def run_on_device(x, target, **run_kwargs):
    """Returns (loss, BassKernelResults)."""
    nc = _get_nc()
    in_maps = _make_in_maps(x, target)
    res = run_bass_kernel_spmd(nc, in_maps, core_ids=list(range(N_CORES)),
                               **run_kwargs)
    loss = _fold([res.results[k]["out"] for k in range(N_CORES)])
    return loss, res


def kernel(x, target):
    loss, _ = run_on_device(x, target)
    return loss


# revision 17
# speedup vs baseline: 1.0806x; 1.0019x over previous
"""Trainium2 Bass kernel for nn_CELossWeighted_28698971472547.

Problem: weighted cross-entropy loss over x[16,32,256,256] logits with
target[16,256,256] class ids; per-pixel weight = 1/(global count of the
pixel's class); loss = sum(ce*w)/sum(w).

Data parallel over 8 NeuronCores (2 images per core). Per core:

  target broadcast (int8-packed, Pool engine):
    targets are packed on host to 1 byte/pixel: for each (tile, group)
    2048-px window, i16 element k = t[px k] | t[px 1024+k] << 8. Four
    tiles x 2048B are concatenated into one 8KB row per (block, group).
    GpSimd partition_broadcast (i32 view) replicates each row to the 32
    class partitions of its group - 16 calls total, cost ~free-size only.

  phase 1, channel-major tiles X [128=(4 groups x 32 ch), 2048 px]:
    ACT   E = exp(X) -> bf16
    DVE   u16[:, 0:1024]   = tb16 & 0xFF   (lo-byte pixels, 4x mode)
    DVE   u16[:, 1024:2048]= tb16 >> 8     (hi-byte pixels, 4x mode)
    DVE   oh = (u16 == iota_col) -> bf16, accum -> per-(g,c) counts (4x)
    DVE   ohe = oh * E (2x)
    PE    per 128-px chunk: lhsT = E-chunk / ohE-chunk [128,128],
          rhs = group-indicator [128,4]
          -> psum[pixel, (chunk, which, group)] = sumexp / exp(x_target)
  phase 2, pixel-major compact [128, 1024]:
    ACT   drain psum; logs = ln(sumexp); lesel = ln(exp(x_t)) ~= x_t
    DVE   v = 64*t + (logs - lesel) = 64*t + ce, stored fp16 (ce in
          [0,~14), v < 2048 so fp16 ulp <= 1; error averages out over
          ~4k px/class, well inside the 2e-2 gate)
    DVE   M_{c+1} = sum_p min(v, 64(c+1)) -- 32 clamp-accumulate passes
          at 4x fp16 rate
  host: fold per-core partials: per-class sums via the telescoping
        identity A_c = M_{c+1} - M_c - 64*N_{>c}; then
        loss = (sum_c A_c/count_c) / #classes-present.

Only lossless layout prep of the integer target happens on host (views,
byte packing of values 0..31) plus the final O(32) fold.
"""

import os
import sys

sys.path.insert(0, "/opt/trn_rl_repo")

ABLATE = int(os.environ.get("ABLATE", "0"))

from contextlib import ExitStack

import numpy as np
import ml_dtypes

import concourse.bass as bass  # noqa: F401
import concourse.tile as tile
from concourse import bacc, mybir
from concourse.bass_utils import run_bass_kernel_spmd

# Pin all activations (Exp/Ln/Copy) to the one table set that contains them
# all, so the table isn't re-loaded between interleaved Exp and Ln batches.
_orig_get_act_tables = bacc.get_activation_tables


def _pinned_act_tables(arch):
    tabs = dict(_orig_get_act_tables(arch))
    AFt = mybir.ActivationFunctionType
    pin = {AFt.Exp, AFt.Ln, AFt.Copy}
    out = {}
    for name, fs in tabs.items():
        if name == "natural_log_exp_and_others":
            out[name] = fs
        else:
            out[name] = fs - pin
    return out


bacc.get_activation_tables = _pinned_act_tables

BF16 = mybir.dt.bfloat16
F16 = mybir.dt.float16
F32 = mybir.dt.float32
I16 = mybir.dt.int16
I32 = mybir.dt.int32
AF = mybir.ActivationFunctionType
ALU = mybir.AluOpType

# ---- problem/shard geometry (hardcoded) ----
N_CORES = 8
C = 32
G = 4
CHUNK = 128
N_IMG = 2              # images per core
HWI = 256 * 256
T = 16                 # tiles per core
F = 2048               # pixels per (tile, group)
HF = F // 2            # pixels per byte lane
NCH = F // CHUNK       # 16 chunks per tile
PM_COLS = T * NCH * G  # 1024
TPB = 4                # tiles per broadcast block
NBLK = T // TPB        # 4 blocks
MSPLITS = 3
SPLITS = [(0, 8), (8, 13), (13, 16)]
MAXMIN_PER_TILE = 8
# h16 extraction engine per tile: True -> ACT (Copy scale=2^-8), False -> DVE
H16_ON_ACT = [t % 4 == 2 for t in range(T)]
# ohe hi-half product engine per tile: True -> Pool (gpsimd TT mult)
OHE_HI_ON_POOL = [True] * T
# ohe lo-half also on Pool for these tiles
OHE_LO_ON_POOL = [False] * T
SHUF_I64 = False


def _build_nc():
    nc = bacc.Bacc("TRN2", target_bir_lowering=False, debug=False,
                   num_devices=N_CORES)
    x_d = nc.dram_tensor("x", [N_IMG, C, HWI], F32, kind="ExternalInput")
    # packed targets: one 8KB row per (group, block), as i32
    tpk_d = nc.dram_tensor("tpk", [G * NBLK, TPB * F // 4], I32,
                           kind="ExternalInput")
    tpm_d = nc.dram_tensor("tpm", [128, PM_COLS], BF16, kind="ExternalInput")
    blk4_d = nc.dram_tensor("blk4", [128, G], BF16, kind="ExternalInput")
    iota_d = nc.dram_tensor("iota", [128, 1], F32, kind="ExternalInput")
    out_d = nc.dram_tensor("out", [128, C + 1], F32, kind="ExternalOutput")

    with tile.TileContext(nc) as tc:
        with ExitStack() as ctx:
            _build_body(ctx, tc, x_d, tpk_d, tpm_d, blk4_d, iota_d, out_d)
    nc.compile()
    return nc


def _build_body(ctx, tc, x_d, tpk_d, tpm_d, blk4_d, iota_d, out_d):
    nc = tc.nc
    xap = x_d.ap()

    consts = ctx.enter_context(tc.tile_pool(name="consts", bufs=1))
    W4 = TPB * F // 4
    tsrc = consts.tile([128, W4], I32)
    for g in range(G):
        nc.sync.dma_start(tsrc[32 * g:32 * g + NBLK, :],
                          tpk_d.ap()[NBLK * g:NBLK * (g + 1), :])
    blk4 = consts.tile([128, G], BF16)
    nc.scalar.dma_start(blk4[:, :], blk4_d.ap())
    iota_col = consts.tile([128, 1], F32)
    nc.scalar.dma_start(iota_col[:, :], iota_d.ap())
    tpm = consts.tile([128, PM_COLS], BF16)

    xpool = ctx.enter_context(tc.tile_pool(name="x", bufs=4))
    tbpool = ctx.enter_context(tc.tile_pool(name="tbp", bufs=2))
    epool = ctx.enter_context(tc.tile_pool(name="e", bufs=4))
    ppool = ctx.enter_context(tc.tile_pool(name="ps", bufs=1, space="PSUM"))

    se = consts.tile([128, 2 * PM_COLS], F32)
    cnt_cols = consts.tile([128, T], F32)
    out_sb = consts.tile([128, C + 1], F32)

    psum = ppool.tile([128, 4096], F32)  # bank t%8 = tile t

    ph2 = ctx.enter_context(tc.tile_pool(name="ph2", bufs=2))
    jpool = ctx.enter_context(tc.tile_pool(name="jp", bufs=2))
    mgr = consts.tile([128, C * MSPLITS], F32)

    prev_mm = None
    tb_blocks = [None] * NBLK
    pending = []
    vv_tiles = [None] * MSPLITS

    I64 = mybir.dt.int64

    def bcast_block(b):
        tb = tbpool.tile([128, W4], I32, tag="tb")
        if SHUF_I64:
            nc.vector.stream_shuffle(tb[:, :].bitcast(I64),
                                     tsrc[:, :].bitcast(I64), [b] * 32)
        else:
            nc.vector.stream_shuffle(tb[:, :], tsrc[:, :], [b] * 32)
        return tb

    tb_blocks[0] = bcast_block(0)

    for t in range(T):
        b, tau = divmod(t, TPB)
        if tau == 0 and b + 1 < NBLK:
            tb_blocks[b + 1] = bcast_block(b + 1)
        if t == 2:
            nc.scalar.dma_start(tpm[:, :], tpm_d.ap())
        n = (G * t * F) // HWI
        off = (G * t * F) % HWI
        xt = xpool.tile([128, F], F32, tag="xt")
        if t == 0:
            xsrc = xap[n][:, off:off + G * F].rearrange("c (g p) -> g c p", g=G)
            nc.sync.dma_start(xt[:, 0:HF], xsrc[:, :, 0:HF])
            nc.sync.dma_start(xt[:, HF:F], xsrc[:, :, HF:F])
        else:
            nc.sync.dma_start(
                xt[:, :],
                xap[n][:, off:off + G * F].rearrange("c (g p) -> g c p", g=G))

        et = epool.tile([128, F], BF16, tag="et")
        if t == 0:
            nc.scalar.activation(et[:, 0:HF], xt[:, 0:HF], AF.Exp)
            nc.scalar.activation(et[:, HF:F], xt[:, HF:F], AF.Exp)
        else:
            nc.scalar.activation(et[:, :], xt[:, :], AF.Exp)

        if ABLATE >= 3:
            continue

        # byte extraction: tb16 element k of this tile = t[px k] | t[px 1024+k]<<8
        tbv = tb_blocks[b][:, tau * (F // 4):(tau + 1) * (F // 4)].bitcast(I16)
        u16 = epool.tile([128, F], I16, tag="u16")
        nc.vector.tensor_scalar(u16[:, 0:HF], tbv, 255, 0,
                                ALU.bitwise_and, ALU.bitwise_or)
        if H16_ON_ACT[t]:
            nc.scalar.activation(u16[:, HF:F], tbv, AF.Copy, scale=0.00390625)
        else:
            nc.vector.tensor_scalar(u16[:, HF:F], tbv, 8, 0,
                                    ALU.logical_shift_right, ALU.bitwise_or)
        oh = epool.tile([128, F], BF16, tag="oh")
        nc.vector.tensor_scalar(oh[:, :], u16[:, :], iota_col[:, 0:1], None,
                                ALU.is_equal, ALU.add,
                                accum_out=cnt_cols[:, t:t + 1])
        ohe = epool.tile([128, F], BF16, tag="ohe")
        lo_eng = nc.gpsimd if OHE_LO_ON_POOL[t] else nc.vector
        if OHE_HI_ON_POOL[t]:
            lo_eng.tensor_tensor(ohe[:, 0:HF], oh[:, 0:HF], et[:, 0:HF],
                                 ALU.mult)
            nc.gpsimd.tensor_tensor(ohe[:, HF:F], oh[:, HF:F], et[:, HF:F],
                                    ALU.mult)
        else:
            nc.vector.tensor_tensor(ohe[:, :], oh[:, :], et[:, :], ALU.mult)

        if ABLATE >= 2:
            continue
        for ch in range(NCH):
            base = (t % 8) * 512 + ch * 2 * G
            sl = slice(CHUNK * ch, CHUNK * (ch + 1))
            for (lo, src) in ((base, et), (base + G, ohe)):
                first = lo % 512 == 0
                last = lo % 512 == 504
                mm = nc.tensor.matmul(psum[:, lo:lo + G], src[:, sl],
                                      blk4[:, :], start=first, stop=last,
                                      skip_group_check=True)
                if prev_mm is not None:
                    tile.add_dep_helper(mm.ins, prev_mm.ins, sync=False,
                                        reason="psum bank program order")
                prev_mm = mm

        for dt in ([t - 1] if t < T - 1 else [t - 1, t]):
            if dt < 0:
                continue
            dsl = slice(dt * NCH * 2 * G, (dt + 1) * NCH * 2 * G)
            nc.scalar.activation(
                se[:, dsl],
                psum[:, (dt % 8) * 512:(dt % 8) * 512 + 128], AF.Copy)

        # incremental phase 2: prep each split when its drains are emitted,
        # then spread the 32 min-accum passes across later tile iterations
        done_ts = [t - 1] if t < T - 1 else [t - 1, t]
        ready = [i for i, (a, bb) in enumerate(SPLITS) if bb - 1 in done_ts]
        for sp in (ready if ABLATE < 1 else []):
            a, bb = SPLITS[sp]
            wt = PM_COLS // T
            w = (bb - a) * wt
            pm_sl = slice(a * wt, bb * wt)
            se_sl = se[:, 2 * a * wt: 2 * bb * wt]
            se_v = se_sl.rearrange("p (a w g) -> p a w g", w=2, g=G)
            logs = ph2.tile([128, w], F32, tag="logs")
            lesel = ph2.tile([128, w], F32, tag="lesel")
            nc.scalar.activation(logs[:, :].rearrange("p (a g) -> p a g", g=G),
                                 se_v[:, :, 0, :], AF.Ln)
            nc.scalar.activation(lesel[:, :].rearrange("p (a g) -> p a g", g=G),
                                 se_v[:, :, 1, :], AF.Ln)
            v1 = ph2.tile([128, w], F32, tag="v1")
            nc.vector.scalar_tensor_tensor(v1[:, :], tpm[:, pm_sl], 64.0,
                                           logs[:, :], ALU.mult, ALU.add)
            vv = ph2.tile([128, w], F16, tag=f"vv{sp}")
            nc.vector.tensor_tensor(vv[:, :], v1[:, :], lesel[:, :],
                                    ALU.subtract)
            vv_tiles[sp] = vv
            pending.extend((sp, c) for c in range(C))
        nmin = len(pending) if t == T - 1 else min(MAXMIN_PER_TILE,
                                                  len(pending))
        for _ in range(nmin):
            sp, c = pending.pop(0)
            a, bb = SPLITS[sp]
            w = (bb - a) * (PM_COLS // T)
            junk = jpool.tile([128, w], F16, tag="junk")
            nc.vector.tensor_scalar(
                junk[:, :], vv_tiles[sp][:, :], float(64 * (c + 1)), None,
                ALU.min, ALU.add,
                accum_out=mgr[:, c * MSPLITS + sp:c * MSPLITS + sp + 1])

    if ABLATE < 1:
        nc.vector.tensor_reduce(out_sb[:, 0:C],
                                mgr[:, :].rearrange("p (c s) -> p c s", s=MSPLITS),
                                mybir.AxisListType.X, ALU.add)
    else:
        nc.vector.memset(out_sb[:, 0:C], 0.0)
    nc.vector.tensor_reduce(out_sb[:, C:C + 1], cnt_cols[:, :],
                            mybir.AxisListType.X, ALU.add)
    nc.sync.dma_start(out_d.ap(), out_sb[:, :])


# ---- host side ----
def _pm_index():
    t_i, ch_i, g_i = np.meshgrid(np.arange(T), np.arange(NCH), np.arange(G),
                                 indexing="ij")
    col_pix = ((G * t_i + g_i) * F + CHUNK * ch_i).reshape(-1)
    return col_pix[None, :] + np.arange(CHUNK)[:, None]   # [128, PM_COLS]


_PM_IDX = _pm_index()
_BLK4 = np.zeros((128, G), dtype=ml_dtypes.bfloat16)
for _g in range(G):
    _BLK4[C * _g:C * (_g + 1), _g] = 1
_IOTA = np.tile(np.arange(C), G).reshape(128, 1).astype(np.float32)

_NC_CACHE = [None]


def _get_nc():
    if _NC_CACHE[0] is None:
        _NC_CACHE[0] = _build_nc()
    return _NC_CACHE[0]


def _pack_targets(tk):
    """tk: flat [2*HWI] int array for this core -> [G*NBLK, TPB*F//4] i32.

    Row (g, b) = concat over tau in [0,TPB) of the 2048-byte packed row
    for tile (TPB*b+tau), group g; within a tile-row, byte 2k holds
    t[px k] and byte 2k+1 holds t[px 1024+k]. Rows are ordered g-major
    so row (g, b) DMAs to SBUF partition 32*g+b for stream_shuffle."""
    tw = tk.reshape(T, G, F).astype(np.uint8)       # [t, g, px]
    packed = np.zeros((T, G, F), np.uint8)
    packed[:, :, 0::2] = tw[:, :, 0:HF]
    packed[:, :, 1::2] = tw[:, :, HF:F]
    # [t, g, F] -> [g, b, tau, F] -> rows (g*NBLK+b, tau*F)
    pb = packed.reshape(NBLK, TPB, G, F).transpose(2, 0, 1, 3)
    return np.ascontiguousarray(pb.reshape(G * NBLK, TPB * F)).view(np.int32)


def _make_in_maps(x, target):
    xs = np.asarray(x, dtype=np.float32).reshape(16, C, HWI)
    tf = np.asarray(target).reshape(16, HWI).astype(np.int32)
    in_maps = []
    for k in range(N_CORES):
        xk = np.ascontiguousarray(xs[2 * k:2 * k + 2])
        tk = np.ascontiguousarray(tf[2 * k:2 * k + 2]).reshape(-1)
        in_maps.append({
            "x": xk,
            "tpk": _pack_targets(tk),
            "tpm": tk[_PM_IDX].astype(ml_dtypes.bfloat16),
            "blk4": _BLK4,
            "iota": _IOTA,
        })
    return in_maps


def _fold(outs):
    M = np.zeros(C + 1, np.float64)   # M[0] = 0; M[j] = sum min(v, 64j)
    cnt = np.zeros(C, np.float64)
    for o in outs:
        o = np.asarray(o, dtype=np.float64)
        M[1:] += o[:, :C].sum(axis=0)
        cnt += o[:, C].reshape(G, C).sum(axis=0)
    n_gt = np.concatenate([np.cumsum(cnt[::-1])[::-1][1:], [0.0]])  # N_{>c}
    A = M[1:] - M[:-1] - 64.0 * n_gt
    present = cnt > 0
    num = (A[present] / cnt[present]).sum()
    den = float(present.sum())
    return np.float32(num / den)


def run_on_device(x, target, **run_kwargs):
    """Returns (loss, BassKernelResults)."""
    nc = _get_nc()
    in_maps = _make_in_maps(x, target)
    res = run_bass_kernel_spmd(nc, in_maps, core_ids=list(range(N_CORES)),
                               **run_kwargs)
    loss = _fold([res.results[k]["out"] for k in range(N_CORES)])
    return loss, res


def kernel(x, target):
    loss, _ = run_on_device(x, target)
    return loss
